# revision 1
# baseline (speedup 1.0000x reference)
"""Kernel builder for causal self-attention (RoPE + parameter-free RMSNorm on Q/K).

Sharding: 8 cores = 4 batch x 2 head-groups (8 heads each). Each core computes
its batch element's attention for its 8 heads plus the partial output
projection; host sums the two head-group partials per batch element.

Per-core device layout (D=64, 8 heads):
  Q^T / K^T stored as [128, 4, T]: col j = 128*cc + p,
     cc = 2*(h//4) + half, p = 32*(h%4) + r,  (d = 32*half + r)
  V stored with a ones column per head: [128, T//128, 8*65]; the ones column
  makes the PV matmul also accumulate the softmax denominator (row 64).
  Scores computed transposed: S^T[tk, tq] per head via K=32 row-tiled matmuls;
  softmax runs without max-subtraction (RMS-normed q,k bound |s| <= 8);
  the denominator division folds in before the output projection.
"""

import sys

import numpy as np

for _p in ("/opt/trn_rl_repo",):
    if _p not in sys.path:
        sys.path.insert(0, _p)

import concourse.bass as bass
import concourse.mybir as mybir
import concourse.tile as tile
from concourse import bacc

F32 = mybir.dt.float32
F32R = mybir.dt.float32r
AX = mybir.AluOpType
ACTF = mybir.ActivationFunctionType

D = 64
NH = 8          # heads per core
CH = NH * D     # 512 head channels per core
EPS = float(np.finfo(np.float32).eps)


def qk_col_perm():
    """perm[j] = plain column (64*h + d) stored at device column j."""
    perm = np.zeros(CH, dtype=np.int64)
    for h in range(NH):
        for half in range(2):
            for r in range(32):
                j = 128 * (2 * (h // 4) + half) + 32 * (h % 4) + r
                perm[j] = 64 * h + 32 * half + r
    return perm


def make_consts(T):
    """Host-side constant tensors fed as kernel inputs."""
    cs_d = D // 2
    inv_freq = 1.0 / (10000.0 ** (np.arange(cs_d, dtype=np.float64) / cs_d))
    freqs = np.outer(np.arange(T, dtype=np.float64), inv_freq)  # [T, 32]
    cosT = np.cos(freqs).astype(np.float32).T  # [32, T]
    sinT = np.sin(freqs).astype(np.float32).T
    COS = np.tile(cosT, (4, 1))  # [128, T]
    SIN = np.tile(sinT, (4, 1))
    # Boundary mask strip [128, 128]: MASK[p, j] = (p <= j)
    p = np.arange(128)[:, None]
    j = np.arange(128)[None, :]
    MASK = (p <= j).astype(np.float32)
    # SEL for ssq reduction: SEL_g[p, m] = 1 if m == 4*g + p//32  ([128, 8])
    SELA = np.zeros((128, 8), dtype=np.float32)
    SELB = np.zeros((128, 8), dtype=np.float32)
    for pp in range(128):
        SELA[pp, pp // 32] = 1.0
        SELB[pp, 4 + pp // 32] = 1.0
    SELTA = SELA.T.copy()
    SELTB = SELB.T.copy()
    ONESF = np.ones((128, 128), dtype=np.float32)
    return dict(COS=COS, SIN=SIN, MASK=MASK, SELA=SELA, SELB=SELB,
                SELTA=SELTA, SELTB=SELTB, ONESF=ONESF)


def make_core_inputs(x_b, Wq_s, Wk_s, Wv_s, Wo_s, consts):
    """x_b [T, CIN]; W*_s are this core's shards: Wq/Wk/Wv [CIN, 512] (plain
    column order 64h+d), Wo_s [512, COUT]. Returns the kernel input map."""
    perm = qk_col_perm()
    return dict(
        xT=np.ascontiguousarray(x_b.T),
        Wq=np.ascontiguousarray(Wq_s[:, perm]),
        Wk=np.ascontiguousarray(Wk_s[:, perm]),
        Wv=np.ascontiguousarray(Wv_s),
        Wo=np.ascontiguousarray(Wo_s),
        **{k: np.ascontiguousarray(v) for k, v in consts.items()},
    )


def build_nc(T, CIN, COUT):
    """Build the Bass program. T seq len, CIN input channels, COUT out channels."""
    assert T % 512 == 0 and CIN % 128 == 0 and COUT % 512 == 0
    KC = CIN // 128        # c_in chunks
    NTB = T // 512         # projection t-blocks == tq blocks
    NQ = T // 512
    NKC = T // 128         # tk chunks
    NCO = COUT // 512      # out-proj column halves

    nc = bacc.Bacc()

    xT = nc.dram_tensor("xT", [CIN, T], F32, kind="ExternalInput")
    Wq = nc.dram_tensor("Wq", [CIN, CH], F32, kind="ExternalInput")
    Wk = nc.dram_tensor("Wk", [CIN, CH], F32, kind="ExternalInput")
    Wv = nc.dram_tensor("Wv", [CIN, CH], F32, kind="ExternalInput")
    Wo = nc.dram_tensor("Wo", [CH, COUT], F32, kind="ExternalInput")
    COS = nc.dram_tensor("COS", [128, T], F32, kind="ExternalInput")
    SIN = nc.dram_tensor("SIN", [128, T], F32, kind="ExternalInput")
    MASK = nc.dram_tensor("MASK", [128, 128], F32, kind="ExternalInput")
    SELA = nc.dram_tensor("SELA", [128, 8], F32, kind="ExternalInput")
    SELB = nc.dram_tensor("SELB", [128, 8], F32, kind="ExternalInput")
    SELTA = nc.dram_tensor("SELTA", [8, 128], F32, kind="ExternalInput")
    SELTB = nc.dram_tensor("SELTB", [8, 128], F32, kind="ExternalInput")
    ONESF = nc.dram_tensor("ONESF", [128, 128], F32, kind="ExternalInput")
    OUT = nc.dram_tensor("OUT", [T, COUT], F32, kind="ExternalOutput")

    xT3 = xT.ap().rearrange("(ko ki) t -> ki ko t", ki=128)      # [128, KC, T]
    Wq3 = Wq.ap().rearrange("(ko ki) m -> ki ko m", ki=128)      # [128, KC, 512]
    Wk3 = Wk.ap().rearrange("(ko ki) m -> ki ko m", ki=128)
    Wv3 = Wv.ap().rearrange("(ko ki) m -> ki ko m", ki=128)
    Wo3 = Wo.ap().rearrange("(mo mi) n -> mi mo n", mi=128)      # [128, 4, COUT]

    with tile.TileContext(nc) as tc:
        with (
            tc.tile_pool(name="consts", bufs=1) as cpool,
            tc.tile_pool(name="big", bufs=1) as big,
            tc.tile_pool(name="w", bufs=1) as wpool,
            tc.tile_pool(name="xtb", bufs=2) as xpool,
            tc.tile_pool(name="work", bufs=2) as work,
            tc.tile_pool(name="tmp", bufs=2) as tmp,
            tc.tile_pool(name="dram", bufs=1, space="DRAM") as dpool,
            tc.tile_pool(name="psa", bufs=1, space="PSUM") as psa,
            tc.tile_pool(name="psb", bufs=1, space="PSUM") as psb,
            tc.tile_pool(name="psy", bufs=4, space="PSUM") as psy,
        ):
            # ---- constants ----
            mask_sb = cpool.tile([128, 128], F32, tag="mask")
            nc.sync.dma_start(out=mask_sb, in_=MASK[:, :])
            sela_sb = cpool.tile([128, 8], F32R, tag="sela")
            nc.sync.dma_start(out=sela_sb, in_=SELA[:, :].bitcast(F32R))
            selb_sb = cpool.tile([128, 8], F32R, tag="selb")
            nc.sync.dma_start(out=selb_sb, in_=SELB[:, :].bitcast(F32R))
            selta_sb = cpool.tile([8, 128], F32R, tag="selta")
            nc.sync.dma_start(out=selta_sb, in_=SELTA[:, :].bitcast(F32R))
            seltb_sb = cpool.tile([8, 128], F32R, tag="seltb")
            nc.sync.dma_start(out=seltb_sb, in_=SELTB[:, :].bitcast(F32R))
            ones_sb = cpool.tile([128, 64], F32R, tag="ones")
            nc.sync.dma_start(out=ones_sb, in_=ONESF[:, 0:64].bitcast(F32R))
            biasq = cpool.tile([8, 1], F32, tag="biasq")
            nc.vector.memset(biasq, 64.0 * EPS)
            biask = cpool.tile([8, 1], F32, tag="biask")
            nc.vector.memset(biask, EPS)

            khat = big.tile([128, 4, T], F32R, tag="khat")
            vsb = big.tile([128, NKC, 520], F32R, tag="v")
            vsb4 = vsb.rearrange("p n (h e) -> p n h e", e=65)

            def project_qk(w_sb, xtb, dst, ts, is_q, cos_ts=None):
                """Project one 512-t block into dst[:, :, ts] with RoPE+RMS.
                cos_ts: global t slice for the RoPE tables (defaults to ts)."""
                if cos_ts is None:
                    cos_ts = ts
                cos_sb = work.tile([128, 512], F32, tag="cos")
                nc.sync.dma_start(out=cos_sb, in_=COS[:, cos_ts])
                sin_sb = work.tile([128, 512], F32, tag="sin")
                nc.sync.dma_start(out=sin_sb, in_=SIN[:, cos_ts])
                qpa = psa.tile([128, 2, 512], F32, tag="pa", name="qpa")
                qpb = psb.tile([128, 2, 512], F32, tag="pb", name="qpb")
                for cc in range(4):
                    qp_t = qpa if cc < 2 else qpb
                    for k in range(KC):
                        nc.tensor.matmul(
                            qp_t[:, cc % 2, :],
                            w_sb[:, k, 128 * cc:128 * (cc + 1)],
                            xtb[:, k, :],
                            start=(k == 0), stop=(k == KC - 1),
                        )
                # stage to SBUF on ACT (Copy lives in every table set), so
                # rope runs all-SBUF on DVE and PSUM frees early
                qs = tmp.tile([128, 4, 512], F32R, tag="qs", bufs=1)
                nc.scalar.activation(qs[:, 0:2, :], qpa, ACTF.Copy)
                nc.scalar.activation(qs[:, 2:4, :], qpb, ACTF.Copy)
                # unscaled rope into dst (scaled afterwards, once rms known)
                u1 = qs[:, 0::2, :]
                u2 = qs[:, 1::2, :]
                cosb = cos_sb[:, None, :].to_broadcast([128, 2, 512])
                sinb = sin_sb[:, None, :].to_broadcast([128, 2, 512])
                e1 = tmp.tile([128, 2, 512], F32, tag="r512", bufs=2)
                e2 = tmp.tile([128, 2, 512], F32, tag="r512", bufs=2)
                nc.vector.tensor_mul(e1, u1, cosb)
                nc.vector.tensor_mul(e2, u2, sinb)
                nc.vector.tensor_add(dst[:, 0::2, ts], e1, e2)
                e3 = tmp.tile([128, 2, 512], F32, tag="r512", bufs=2)
                e4 = tmp.tile([128, 2, 512], F32, tag="r512", bufs=2)
                nc.vector.tensor_mul(e3, u2, cosb)
                nc.vector.tensor_mul(e4, u1, sinb)
                nc.vector.tensor_sub(dst[:, 1::2, ts], e3, e4)
                # per-head sum of squares (pre-rope == post-rope norms)
                qsq = tmp.tile([128, 4, 512], F32R, tag="qsq", bufs=1)
                nc.vector.tensor_mul(qsq, qs, qs)
                ssq = psy.tile([8, 512], F32, tag="y", name="ssq")
                for cc in range(4):
                    nc.tensor.matmul(
                        ssq,
                        sela_sb if cc < 2 else selb_sb,
                        qsq[:, cc, :],
                        start=(cc == 0), stop=(cc == 3),
                    )
                # rms factor rows [8, 512]: reciprocal(sqrt(.)). All of
                # phase A uses only {Copy, Sqrt} so one table set suffices.
                sq = tmp.tile([8, 512], F32, tag="sq")
                if is_q:  # 1/sqrt(ssq + 64 eps): folds the 1/sqrt(D) scale
                    nc.scalar.activation(sq, ssq, ACTF.Sqrt, bias=biasq,
                                         scale=1.0)
                else:     # 1/sqrt(ssq/64 + eps)
                    nc.scalar.activation(sq, ssq, ACTF.Sqrt, bias=biask,
                                         scale=1.0 / 64.0)
                rr = tmp.tile([8, 512], F32R, tag="rr")
                with nc.allow_low_precision(reason="f32r feed to PE broadcast"):
                    nc.vector.reciprocal(rr, sq)
                # rms scale applied to the roped output, per chunk pair
                for pr in range(2):
                    bq = psy.tile([128, 512], F32, tag="y", name=f"bq{pr}")
                    nc.tensor.matmul(
                        bq, selta_sb if pr == 0 else seltb_sb, rr,
                        start=True, stop=True,
                    )
                    nc.vector.tensor_mul(
                        dst[:, 2 * pr:2 * pr + 2, ts],
                        dst[:, 2 * pr:2 * pr + 2, ts],
                        bq[:, None, :].to_broadcast([128, 2, 512]),
                    )

            # ============ Phase A: Q-hat->DRAM, K-hat, V (per t-block) ===
            nc.sync.dma_start(
                out=vsb4[:, :, :, 64],
                in_=ONESF.ap()[:, 0:8 * NKC].rearrange(
                    "p (n h) -> p n h", h=8).bitcast(F32R))
            qdram = dpool.tile([128, 4, T], F32R, tag="qd")
            wq_sb = wpool.tile([128, KC, 512], F32R, tag="wa")
            nc.sync.dma_start(out=wq_sb, in_=Wq3[:, :, :].bitcast(F32R))
            wv_sb = wpool.tile([128, KC, 512], F32R, tag="wb")
            nc.sync.dma_start(out=wv_sb, in_=Wv3[:, :, :].bitcast(F32R))
            wk_sb = wpool.tile([128, KC, 512], F32R, tag="wc")
            nc.sync.dma_start(out=wk_sb, in_=Wk3[:, :, :].bitcast(F32R))
            for tb in range(NTB):
                ts = slice(512 * tb, 512 * (tb + 1))
                xtb = xpool.tile([128, KC, 512], F32R, tag="xtb")
                nc.sync.dma_start(out=xtb, in_=xT3[:, :, ts].bitcast(F32R))
                qstage = work.tile([128, 4, 512], F32R, tag="qtb", bufs=1)
                project_qk(wq_sb, xtb, qstage, slice(0, 512), is_q=True,
                           cos_ts=ts)
                nc.sync.dma_start(out=qdram[:, :, ts], in_=qstage)
                project_qk(wk_sb, xtb, khat, ts, is_q=False)
                for j in range(4):
                    vp = psy.tile([128, 512], F32, tag="y", name=f"vp{tb}_{j}")
                    for k in range(KC):
                        nc.tensor.matmul(
                            vp,
                            xtb[:, k, 128 * j:128 * (j + 1)],
                            wv_sb[:, k, :],
                            start=(k == 0), stop=(k == KC - 1),
                        )
                    nc.scalar.activation(
                        vsb4[:, 4 * tb + j, :, 0:64],
                        vp.rearrange("p (h d) -> p h d", d=64), ACTF.Copy)

            # ============ Phase B: per tq block: attention + out-proj ====
            wo_sb = wpool.tile([128, 4, COUT], F32R, tag="wb")
            nc.sync.dma_start(out=wo_sb, in_=Wo3[:, :, :].bitcast(F32R))

            for qb in range(NQ):
                tqs = slice(512 * qb, 512 * (qb + 1))
                qtb = work.tile([128, 4, 512], F32R, tag="qtb", bufs=1)
                nc.sync.dma_start(out=qtb, in_=qdram[:, :, tqs])

                yhat = work.tile([128, 4, 512], F32R, tag="yhat", bufs=1)
                for g in range(2):
                    ybank = [psy.tile([65, 512], F32, tag="y",
                                      name=f"y{qb}_{g}_{j_}") for j_ in range(4)]
                    nkc = 4 * (qb + 1)
                    for c in range(nkc):
                        scs = [psa.tile([128, 2, 512], F32, tag="pa", name="scA"),
                               psb.tile([128, 2, 512], F32, tag="pb", name="scB")]
                        for j in range(4):
                            for half in range(2):
                                cc = 2 * g + half
                                nc.tensor.matmul(
                                    scs[j // 2][:, j % 2, :],
                                    khat[32 * j:32 * (j + 1), cc,
                                         128 * c:128 * (c + 1)],
                                    qtb[32 * j:32 * (j + 1), cc, :],
                                    start=(half == 0), stop=(half == 1),
                                    tile_position=(32 * j, 0),
                                )
                        kd = c - 4 * qb
                        first, last = (c == 0), (c == nkc - 1)
                        # for diagonal chunks only columns [128*kd, 512)
                        # are live: exp, mask, and PV all restrict to the
                        # suffix, so the masked prefix is never touched.
                        # (tq columns < 128*kd take no contribution from this
                        # chunk; chunk 0 is always full-width with start=True.)
                        lo = 128 * kd if kd > 0 else 0
                        for pj in range(2):
                            ph = tmp.tile([128, 2, 512], F32R, tag="r512",
                                          bufs=2, name=f"ph{pj}")
                            nc.scalar.activation(
                                ph[:, :, lo:], scs[pj][:, :, lo:], ACTF.Exp)
                            if kd >= 0:  # diagonal boundary strip
                                nc.vector.tensor_mul(
                                    ph[:, :, 128 * kd:128 * (kd + 1)],
                                    ph[:, :, 128 * kd:128 * (kd + 1)],
                                    mask_sb[:, None, :].to_broadcast(
                                        [128, 2, 128]),
                                )
                            for e in range(2):
                                j = 2 * pj + e
                                hloc = 4 * g + j
                                nc.tensor.matmul(
                                    ybank[j][:, lo:],
                                    vsb[:, c, 65 * hloc:65 * hloc + 65],
                                    ph[:, e, lo:],
                                    start=first, stop=last,
                                    skip_group_check=True,
                                )
                    # normalize: yhat rows = y / denom
                    for j in range(4):
                        hloc = 4 * g + j
                        rcp = tmp.tile([128, 512], F32R, tag="s512")
                        with nc.allow_low_precision(reason="f32r for PE bcast"):
                            nc.vector.reciprocal(rcp[64:65, :],
                                                 ybank[j][64:65, :])
                        rb = psa.tile([128, 512], F32, tag="pa",
                                      name=f"rb{qb}_{g}_{j}")
                        nc.tensor.matmul(
                            rb[0:64, :],
                            ones_sb[64:65, :],
                            rcp[64:65, :],
                            start=True, stop=True,
                            tile_position=(64, 0),
                            skip_group_check=True,
                        )
                        rbs = tmp.tile([128, 512], F32, tag="s512")
                        nc.vector.tensor_copy(out=rbs[0:64, :], in_=rb[0:64, :])
                        nc.vector.tensor_mul(
                            yhat[64 * (hloc % 2):64 * (hloc % 2 + 1),
                                 hloc // 2, :],
                            ybank[j][0:64, :],
                            rbs[0:64, :],
                        )
                # out projection for this tq block
                for n in range(NCO):
                    for jt in range(4):
                        op = psy.tile([128, 512], F32, tag="y",
                                      name=f"op{qb}_{n}_{jt}")
                        for m in range(4):
                            nc.tensor.matmul(
                                op,
                                yhat[:, m, 128 * jt:128 * (jt + 1)],
                                wo_sb[:, m, 512 * n:512 * (n + 1)],
                                start=(m == 0), stop=(m == 3),
                            )
                        osb = tmp.tile([128, 512], F32, tag="s512")
                        nc.vector.tensor_copy(out=osb, in_=op)
                        nc.sync.dma_start(
                            out=OUT[512 * qb + 128 * jt:512 * qb + 128 * (jt + 1),
                                    512 * n:512 * (n + 1)],
                            in_=osb)

    nc.finalize()
    return nc


# ======================================================================
# Full-problem harness: 8 cores = 4 batch x 2 head-groups
# ======================================================================
B_FULL, T_FULL, C_FULL, H_FULL = 4, 2048, 1024, 16

_NC_CACHE = {}


def _get_nc():
    if "nc" not in _NC_CACHE:
        _NC_CACHE["nc"] = build_nc(T_FULL, C_FULL, C_FULL)
    return _NC_CACHE["nc"]


def _consts_from_tables(cos, sin):
    """Like make_consts but using the provided RoPE tables.
    cos/sin: [1, 1, T, 32] float32."""
    c = make_consts(T_FULL)
    c["COS"] = np.ascontiguousarray(np.tile(np.asarray(cos)[0, 0].T, (4, 1)))
    c["SIN"] = np.ascontiguousarray(np.tile(np.asarray(sin)[0, 0].T, (4, 1)))
    return c


def make_in_maps(x, cos, sin, Wq, Wk, Wv, Wo):
    x, Wq, Wk, Wv, Wo = (np.asarray(a, dtype=np.float32)
                         for a in (x, Wq, Wk, Wv, Wo))
    consts = _consts_from_tables(cos, sin)
    in_maps = []
    for core in range(8):
        b, hg = core // 2, core % 2
        cols = slice(512 * hg, 512 * (hg + 1))
        in_maps.append(make_core_inputs(
            x[b], Wq[:, cols], Wk[:, cols], Wv[:, cols], Wo[cols, :], consts))
    return in_maps


def gather_out(results):
    out = np.empty((B_FULL, T_FULL, C_FULL), dtype=np.float32)
    for b in range(B_FULL):
        out[b] = results[2 * b]["OUT"] + results[2 * b + 1]["OUT"]
    return out


def kernel(x, cos, sin, Wq, Wk, Wv, Wo):
    from concourse.bass_utils import run_bass_kernel_spmd
    nc = _get_nc()
    in_maps = make_in_maps(x, cos, sin, Wq, Wk, Wv, Wo)
    res = run_bass_kernel_spmd(nc, in_maps, core_ids=list(range(8)))
    return gather_out(res.results)



# revision 2
# speedup vs baseline: 5.5426x; 5.5426x over previous
"""Causal self-attention kernel (RoPE + parameter-free RMSNorm on Q/K).

Sharding: 8 cores = 4 batch x 2 head-groups (8 heads each). Each core computes
its batch element's attention for its 8 heads plus the partial output
projection; host sums the two head-group partials per batch element.

All per-core inputs ship as ONE packed bf16 blob [128, NCOL] (x, weights, RoPE
tables, mask/select constants) to minimize per-call host->device traffic and
buffer-binding overhead on the axon exec path. All matmuls run bf16 with f32
PSUM accumulation; the output ships back as bf16 partials summed on host in
f32. Q-hat stays resident in SBUF (no DRAM round trip).

Per-core device layout (D=64, 8 heads):
  Q^T / K^T stored as [128, 4, T]: col j = 128*cc + p,
     cc = 2*(h//4) + half, p = 32*(h%4) + r,  (d = 32*half + r)
  V stored with a ones column per head: [128, T//128, 8*65]; the ones column
  makes the PV matmul also accumulate the softmax denominator (row 64).
  Scores computed transposed: S^T[tk, tq] per head via K=32 row-tiled matmuls;
  softmax runs without max-subtraction (RMS-normed q,k bound |s| <= 8);
  the denominator division folds in before the output projection.
"""

import sys

import numpy as np
import ml_dtypes

for _p in ("/opt/trn_rl_repo",):
    if _p not in sys.path:
        sys.path.insert(0, _p)

import concourse.bass as bass
import concourse.mybir as mybir
import concourse.tile as tile
from concourse import bacc

F32 = mybir.dt.float32
BF16 = mybir.dt.bfloat16
AX = mybir.AluOpType
ACTF = mybir.ActivationFunctionType
BF_NP = ml_dtypes.bfloat16

D = 64
NH = 8          # heads per core
CH = NH * D     # 512 head channels per core
EPS = float(np.finfo(np.float32).eps)


def blob_layout(T, CIN=1024, COUT=1024):
    """Column offsets of each section in the packed [128, NCOL] bf16 blob."""
    KC = CIN // 128
    o = {}
    c = 0
    o["X"] = c; c += KC * T            # [128, KC, T]
    o["WQ"] = c; c += KC * CH          # [128, KC, 512]
    o["WK"] = c; c += KC * CH
    o["WV"] = c; c += KC * CH
    o["WO"] = c; c += 4 * COUT         # [128, 4, COUT]
    o["COS"] = c; c += T               # [128, T]
    o["SIN"] = c; c += T
    o["MASK"] = c; c += 128            # [128, 128]
    o["SELA"] = c; c += 8              # [128, 8]
    o["SELB"] = c; c += 8
    o["SELTA"] = c; c += 128           # rows 0:8 used, [8, 128]
    o["SELTB"] = c; c += 128
    o["NCOL"] = c
    return o


def qk_col_perm():
    """perm[j] = plain column (64*h + d) stored at device column j."""
    perm = np.zeros(CH, dtype=np.int64)
    for h in range(NH):
        for half in range(2):
            for r in range(32):
                j = 128 * (2 * (h // 4) + half) + 32 * (h % 4) + r
                perm[j] = 64 * h + 32 * half + r
    return perm


def make_consts(T):
    """Host-side constant tables (f32; rounded to bf16 at blob pack time)."""
    cs_d = D // 2
    inv_freq = 1.0 / (10000.0 ** (np.arange(cs_d, dtype=np.float64) / cs_d))
    freqs = np.outer(np.arange(T, dtype=np.float64), inv_freq)  # [T, 32]
    cosT = np.cos(freqs).astype(np.float32).T  # [32, T]
    sinT = np.sin(freqs).astype(np.float32).T
    COS = np.tile(cosT, (4, 1))  # [128, T]
    SIN = np.tile(sinT, (4, 1))
    p = np.arange(128)[:, None]
    j = np.arange(128)[None, :]
    MASK = (p <= j).astype(np.float32)
    SELA = np.zeros((128, 8), dtype=np.float32)
    SELB = np.zeros((128, 8), dtype=np.float32)
    for pp in range(128):
        SELA[pp, pp // 32] = 1.0
        SELB[pp, 4 + pp // 32] = 1.0
    return dict(COS=COS, SIN=SIN, MASK=MASK, SELA=SELA, SELB=SELB,
                SELTA=SELA.T.copy(), SELTB=SELB.T.copy())


def _chunked_rows(a):
    """[CIN, M] -> [128, CIN//128 * M]: row c = 128*ko + ki lands at
    partition ki, col chunk ko."""
    cin, m = a.shape
    kc = cin // 128
    return np.ascontiguousarray(
        a.reshape(kc, 128, m).transpose(1, 0, 2).reshape(128, kc * m))


def make_core_inputs(x_b, Wq_s, Wk_s, Wv_s, Wo_s, consts):
    """x_b [T, CIN]; W*_s this core's shards: Wq/Wk/Wv [CIN, 512] (plain
    column order 64h+d), Wo_s [512, COUT]. Returns {"BLOB": packed bf16}."""
    T = x_b.shape[0]
    cin = x_b.shape[1]
    cout = Wo_s.shape[1]
    o = blob_layout(T, cin, cout)
    perm = qk_col_perm()
    blob = np.zeros((128, o["NCOL"]), dtype=np.float32)
    blob[:, o["X"]:o["WQ"]] = _chunked_rows(np.ascontiguousarray(x_b.T))
    blob[:, o["WQ"]:o["WK"]] = _chunked_rows(Wq_s[:, perm])
    blob[:, o["WK"]:o["WV"]] = _chunked_rows(Wk_s[:, perm])
    blob[:, o["WV"]:o["WO"]] = _chunked_rows(Wv_s)
    blob[:, o["WO"]:o["COS"]] = _chunked_rows(Wo_s)
    blob[:, o["COS"]:o["SIN"]] = consts["COS"]
    blob[:, o["SIN"]:o["MASK"]] = consts["SIN"]
    blob[:, o["MASK"]:o["SELA"]] = consts["MASK"]
    blob[:, o["SELA"]:o["SELB"]] = consts["SELA"]
    blob[:, o["SELB"]:o["SELTA"]] = consts["SELB"]
    blob[0:8, o["SELTA"]:o["SELTB"]] = consts["SELTA"]
    blob[0:8, o["SELTB"]:o["NCOL"]] = consts["SELTB"]
    return {"BLOB": blob.astype(BF_NP)}


def build_nc(T, CIN, COUT):
    """Build the Bass program. T seq len, CIN input channels, COUT out channels."""
    assert T % 512 == 0 and CIN % 128 == 0 and COUT % 512 == 0
    KC = CIN // 128        # c_in chunks
    NTB = T // 512         # projection t-blocks == tq blocks
    NQ = T // 512
    NKC = T // 128         # tk chunks
    NCO = COUT // 512      # out-proj column halves
    o = blob_layout(T, CIN, COUT)

    nc = bacc.Bacc()
    BLOB = nc.dram_tensor("BLOB", [128, o["NCOL"]], BF16, kind="ExternalInput")
    OUT = nc.dram_tensor("OUT", [T, COUT], BF16, kind="ExternalOutput")

    with tile.TileContext(nc) as tc:
        with (
            tc.tile_pool(name="consts", bufs=1) as cpool,
            tc.tile_pool(name="big", bufs=1) as big,
            tc.tile_pool(name="work", bufs=2) as work,
            tc.tile_pool(name="tmp", bufs=2) as tmp,
            tc.tile_pool(name="psa", bufs=1, space="PSUM") as psa,
            tc.tile_pool(name="psb", bufs=1, space="PSUM") as psb,
            tc.tile_pool(name="psy", bufs=4, space="PSUM") as psy,
        ):
            # ---- one-shot loads from the blob ----
            xfull = big.tile([128, KC * T], BF16, tag="xfull")
            nc.sync.dma_start(out=xfull, in_=BLOB[:, o["X"]:o["X"] + KC * T])
            wtile = big.tile([128, 3 * KC * CH + 4 * COUT], BF16, tag="wtile")
            nc.sync.dma_start(out=wtile, in_=BLOB[:, o["WQ"]:o["COS"]])
            cstile = big.tile([128, 2 * T], BF16, tag="cstile")
            nc.sync.dma_start(out=cstile, in_=BLOB[:, o["COS"]:o["MASK"]])
            ctile = cpool.tile([128, o["NCOL"] - o["MASK"]], BF16, tag="ctile")
            nc.sync.dma_start(out=ctile, in_=BLOB[:, o["MASK"]:o["NCOL"]])

            def xs(k, lo, n):      # x chunk k, t-cols [lo, lo+n)
                return xfull[:, k * T + lo: k * T + lo + n]

            def ws(which, k, lo, n):   # weight section, chunk k, cols [lo,lo+n)
                base = {"WQ": 0, "WK": KC * CH, "WV": 2 * KC * CH,
                        "WO": 3 * KC * CH}[which]
                return wtile[:, base + k * CH + lo: base + k * CH + lo + n]

            def wo_s(m, lo, n):
                return wtile[:, 3 * KC * CH + m * COUT + lo:
                             3 * KC * CH + m * COUT + lo + n]

            mb = o["MASK"]
            mask_sb = ctile[:, 0:128]
            sela_sb = ctile[:, o["SELA"] - mb:o["SELB"] - mb]
            selb_sb = ctile[:, o["SELB"] - mb:o["SELTA"] - mb]
            selta_sb = ctile[0:8, o["SELTA"] - mb:o["SELTB"] - mb]
            seltb_sb = ctile[0:8, o["SELTB"] - mb:o["NCOL"] - mb]

            ones_sb = cpool.tile([128, 64], BF16, tag="ones")
            nc.vector.memset(ones_sb, 1.0)
            biasq = cpool.tile([8, 1], F32, tag="biasq")
            nc.vector.memset(biasq, 64.0 * EPS)
            biask = cpool.tile([8, 1], F32, tag="biask")
            nc.vector.memset(biask, EPS)

            khat = big.tile([128, 4, T], BF16, tag="khat")
            qhat = big.tile([128, 4, T], BF16, tag="qhat")
            vsb = big.tile([128, NKC, 520], BF16, tag="v")
            vsb4 = vsb.rearrange("p n (h e) -> p n h e", e=65)
            nc.vector.memset(vsb4[:, :, :, 64], 1.0)

            def project_qk(which, tb, dst, is_q):
                """Project t-block tb into dst[:, :, ts] with RoPE+RMS."""
                ts = slice(512 * tb, 512 * (tb + 1))
                cos_sb = cstile[:, 512 * tb:512 * (tb + 1)]
                sin_sb = cstile[:, T + 512 * tb:T + 512 * (tb + 1)]
                qpa = psa.tile([128, 2, 512], F32, tag="pa", name="qpa")
                qpb = psb.tile([128, 2, 512], F32, tag="pb", name="qpb")
                for cc in range(4):
                    qp_t = qpa if cc < 2 else qpb
                    for k in range(KC):
                        nc.tensor.matmul(
                            qp_t[:, cc % 2, :],
                            ws(which, k, 128 * cc, 128),
                            xs(k, 512 * tb, 512),
                            start=(k == 0), stop=(k == KC - 1),
                        )
                qs = tmp.tile([128, 4, 512], BF16, tag="qs", bufs=1)
                nc.scalar.activation(qs[:, 0:2, :], qpa, ACTF.Copy)
                nc.scalar.activation(qs[:, 2:4, :], qpb, ACTF.Copy)
                # unscaled rope into dst (scaled afterwards, once rms known)
                u1 = qs[:, 0::2, :]
                u2 = qs[:, 1::2, :]
                cosb = cos_sb[:, None, :].to_broadcast([128, 2, 512])
                sinb = sin_sb[:, None, :].to_broadcast([128, 2, 512])
                e1 = tmp.tile([128, 2, 512], BF16, tag="r512", bufs=2)
                e2 = tmp.tile([128, 2, 512], BF16, tag="r512", bufs=2)
                nc.vector.tensor_mul(e1, u1, cosb)
                nc.vector.tensor_mul(e2, u2, sinb)
                nc.vector.tensor_add(dst[:, 0::2, ts], e1, e2)
                e3 = tmp.tile([128, 2, 512], BF16, tag="r512", bufs=2)
                e4 = tmp.tile([128, 2, 512], BF16, tag="r512", bufs=2)
                nc.vector.tensor_mul(e3, u2, cosb)
                nc.vector.tensor_mul(e4, u1, sinb)
                nc.vector.tensor_sub(dst[:, 1::2, ts], e3, e4)
                # per-head sum of squares (pre-rope == post-rope norms)
                qsq = tmp.tile([128, 4, 512], BF16, tag="qsq", bufs=1)
                nc.vector.tensor_mul(qsq, qs, qs)
                ssq = psy.tile([8, 512], F32, tag="y", name="ssq")
                for cc in range(4):
                    nc.tensor.matmul(
                        ssq,
                        sela_sb if cc < 2 else selb_sb,
                        qsq[:, cc, :],
                        start=(cc == 0), stop=(cc == 3),
                    )
                sq = tmp.tile([8, 512], F32, tag="sq")
                if is_q:  # 1/sqrt(ssq + 64 eps): folds the 1/sqrt(D) scale
                    nc.scalar.activation(sq, ssq, ACTF.Sqrt, bias=biasq,
                                         scale=1.0)
                else:     # 1/sqrt(ssq/64 + eps)
                    nc.scalar.activation(sq, ssq, ACTF.Sqrt, bias=biask,
                                         scale=1.0 / 64.0)
                rr = tmp.tile([8, 512], BF16, tag="rr")
                with nc.allow_low_precision(reason="bf16 feed to PE broadcast"):
                    nc.vector.reciprocal(rr, sq)
                for pr in range(2):
                    bq = psy.tile([128, 512], F32, tag="y", name=f"bq{pr}")
                    nc.tensor.matmul(
                        bq, selta_sb if pr == 0 else seltb_sb, rr,
                        start=True, stop=True,
                    )
                    bqs = tmp.tile([128, 512], BF16, tag="bqs", bufs=2)
                    nc.scalar.activation(bqs, bq, ACTF.Copy)
                    nc.vector.tensor_mul(
                        dst[:, 2 * pr:2 * pr + 2, ts],
                        dst[:, 2 * pr:2 * pr + 2, ts],
                        bqs[:, None, :].to_broadcast([128, 2, 512]),
                    )

            # ============ Phase A: Q-hat, K-hat, V (per t-block) =========
            for tb in range(NTB):
                project_qk("WQ", tb, qhat, is_q=True)
                project_qk("WK", tb, khat, is_q=False)
                for j in range(4):
                    vp = psy.tile([128, 512], F32, tag="y", name=f"vp{tb}_{j}")
                    for k in range(KC):
                        nc.tensor.matmul(
                            vp,
                            xs(k, 512 * tb + 128 * j, 128),
                            ws("WV", k, 0, CH),
                            start=(k == 0), stop=(k == KC - 1),
                        )
                    nc.scalar.activation(
                        vsb4[:, 4 * tb + j, :, 0:64],
                        vp.rearrange("p (h d) -> p h d", d=64), ACTF.Copy)

            # ============ Phase B: per tq block: attention + out-proj ====
            for qb in range(NQ):
                yhat = work.tile([128, 4, 512], BF16, tag="yhat", bufs=1)
                for g in range(2):
                    ybank = [psy.tile([65, 512], F32, tag="y",
                                      name=f"y{qb}_{g}_{j_}") for j_ in range(4)]
                    nkc = 4 * (qb + 1)
                    for c in range(nkc):
                        kd = c - 4 * qb
                        first, last = (c == 0), (c == nkc - 1)
                        # for diagonal chunks only columns [128*kd, 512)
                        # are live: scores, exp, mask, and PV all restrict
                        # to the suffix. (chunk 0 is always full-width.)
                        lo = 128 * kd if kd > 0 else 0
                        scs = [psa.tile([128, 2, 512], F32, tag="pa", name="scA"),
                               psb.tile([128, 2, 512], F32, tag="pb", name="scB")]
                        for j in range(4):
                            for half in range(2):
                                cc = 2 * g + half
                                nc.tensor.matmul(
                                    scs[j // 2][:, j % 2, lo:],
                                    khat[32 * j:32 * (j + 1), cc,
                                         128 * c:128 * (c + 1)],
                                    qhat[32 * j:32 * (j + 1), cc,
                                         512 * qb + lo:512 * (qb + 1)],
                                    start=(half == 0), stop=(half == 1),
                                    tile_position=(32 * j, 0),
                                )
                        for pj in range(2):
                            ph = tmp.tile([128, 2, 512], BF16, tag="r512",
                                          bufs=2, name=f"ph{pj}")
                            nc.scalar.activation(
                                ph[:, :, lo:], scs[pj][:, :, lo:], ACTF.Exp)
                            if kd >= 0:  # diagonal boundary strip
                                nc.vector.tensor_mul(
                                    ph[:, :, 128 * kd:128 * (kd + 1)],
                                    ph[:, :, 128 * kd:128 * (kd + 1)],
                                    mask_sb[:, None, :].to_broadcast(
                                        [128, 2, 128]),
                                )
                            for e in range(2):
                                j = 2 * pj + e
                                hloc = 4 * g + j
                                nc.tensor.matmul(
                                    ybank[j][:, lo:],
                                    vsb[:, c, 65 * hloc:65 * hloc + 65],
                                    ph[:, e, lo:],
                                    start=first, stop=last,
                                    skip_group_check=True,
                                )
                    # normalize: yhat rows = y / denom
                    for j in range(4):
                        hloc = 4 * g + j
                        rcp = tmp.tile([128, 512], BF16, tag="s512")
                        with nc.allow_low_precision(reason="bf16 for PE bcast"):
                            nc.vector.reciprocal(rcp[64:65, :],
                                                 ybank[j][64:65, :])
                        rb = psa.tile([128, 512], F32, tag="pa",
                                      name=f"rb{qb}_{g}_{j}")
                        nc.tensor.matmul(
                            rb[0:64, :],
                            ones_sb[64:65, :],
                            rcp[64:65, :],
                            start=True, stop=True,
                            tile_position=(64, 0),
                            skip_group_check=True,
                        )
                        rbs = tmp.tile([128, 512], F32, tag="s512f")
                        nc.vector.tensor_copy(out=rbs[0:64, :], in_=rb[0:64, :])
                        nc.vector.tensor_mul(
                            yhat[64 * (hloc % 2):64 * (hloc % 2 + 1),
                                 hloc // 2, :],
                            ybank[j][0:64, :],
                            rbs[0:64, :],
                        )
                # out projection for this tq block
                for jt in range(4):
                    osb = tmp.tile([128, COUT], BF16, tag="osb", bufs=2)
                    for n in range(NCO):
                        op = psy.tile([128, 512], F32, tag="y",
                                      name=f"op{qb}_{n}_{jt}")
                        for m in range(4):
                            nc.tensor.matmul(
                                op,
                                yhat[:, m, 128 * jt:128 * (jt + 1)],
                                wo_s(m, 512 * n, 512),
                                start=(m == 0), stop=(m == 3),
                            )
                        nc.vector.tensor_copy(out=osb[:, 512 * n:512 * (n + 1)],
                                              in_=op)
                    nc.sync.dma_start(
                        out=OUT[512 * qb + 128 * jt:512 * qb + 128 * (jt + 1), :],
                        in_=osb)

    nc.finalize()
    return nc


# ======================================================================
# Full-problem harness: 8 cores = 4 batch x 2 head-groups
# ======================================================================
B_FULL, T_FULL, C_FULL, H_FULL = 4, 2048, 1024, 16

_NC_CACHE = {}


def _get_nc():
    if "nc" not in _NC_CACHE:
        _NC_CACHE["nc"] = build_nc(T_FULL, C_FULL, C_FULL)
    return _NC_CACHE["nc"]


def _consts_from_tables(cos, sin):
    """Like make_consts but using the provided RoPE tables.
    cos/sin: [1, 1, T, 32] float32."""
    c = make_consts(T_FULL)
    c["COS"] = np.ascontiguousarray(np.tile(np.asarray(cos)[0, 0].T, (4, 1)))
    c["SIN"] = np.ascontiguousarray(np.tile(np.asarray(sin)[0, 0].T, (4, 1)))
    return c


def make_in_maps(x, cos, sin, Wq, Wk, Wv, Wo):
    x, Wq, Wk, Wv, Wo = (np.asarray(a, dtype=np.float32)
                         for a in (x, Wq, Wk, Wv, Wo))
    consts = _consts_from_tables(cos, sin)
    in_maps = []
    for core in range(8):
        b, hg = core // 2, core % 2
        cols = slice(512 * hg, 512 * (hg + 1))
        in_maps.append(make_core_inputs(
            x[b], Wq[:, cols], Wk[:, cols], Wv[:, cols], Wo[cols, :], consts))
    return in_maps


def gather_out(results):
    out = np.empty((B_FULL, T_FULL, C_FULL), dtype=np.float32)
    for b in range(B_FULL):
        out[b] = (results[2 * b]["OUT"].astype(np.float32)
                  + results[2 * b + 1]["OUT"].astype(np.float32))
    return out


def kernel(x, cos, sin, Wq, Wk, Wv, Wo):
    from concourse.bass_utils import run_bass_kernel_spmd
    nc = _get_nc()
    in_maps = make_in_maps(x, cos, sin, Wq, Wk, Wv, Wo)
    res = run_bass_kernel_spmd(nc, in_maps, core_ids=list(range(8)))
    return gather_out(res.results)


# revision 14
# speedup vs baseline: 8.9070x; 1.6070x over previous
"""Causal self-attention kernel (RoPE + parameter-free RMSNorm on Q/K).

Sharding: 8 cores = 4 batch x 2 head-groups (8 heads each). Each core computes
its batch element's attention for its 8 heads plus the partial output
projection; host sums the two head-group partials per batch element.

All per-core inputs ship as ONE packed bf16 blob [128, NCOL] (x, weights, RoPE
tables, mask/select constants) to minimize per-call host->device traffic and
buffer-binding overhead on the axon exec path. All matmuls run bf16 with f32
PSUM accumulation; the output ships back as bf16 partials summed on host in
f32. Q-hat stays resident in SBUF (no DRAM round trip).

Per-core device layout (D=64, 8 heads):
  Q^T / K^T stored as [128, 4, T]: col j = 128*cc + p,
     cc = 2*(h//4) + half, p = 32*(h%4) + r,  (d = 32*half + r)
  V stored with a ones column per head: [128, T//128, 8*65]; the ones column
  makes the PV matmul also accumulate the softmax denominator (row 64).
  Scores computed transposed: S^T[tk, tq] per head via K=32 row-tiled matmuls;
  softmax runs without max-subtraction (RMS-normed q,k bound |s| <= 8);
  the denominator division folds in before the output projection.
"""

import sys

import numpy as np
import ml_dtypes

for _p in ("/opt/trn_rl_repo",):
    if _p not in sys.path:
        sys.path.insert(0, _p)

import concourse.bass as bass
import concourse.mybir as mybir
import concourse.tile as tile
from concourse import bacc

F32 = mybir.dt.float32
BF16 = mybir.dt.bfloat16
AX = mybir.AluOpType
ACTF = mybir.ActivationFunctionType
BF_NP = ml_dtypes.bfloat16

D = 64
NH = 8          # heads per core
CH = NH * D     # 512 head channels per core
EPS = float(np.finfo(np.float32).eps)


def blob_layout(T, CIN=1024, COUT=1024):
    """Column offsets of each section in the packed [128, NCOL] bf16 blob."""
    KC = CIN // 128
    o = {}
    c = 0
    o["X"] = c; c += KC * T            # [128, KC, T]
    o["WQ"] = c; c += KC * CH          # [128, KC, 512]
    o["WK"] = c; c += KC * CH
    o["WV"] = c; c += KC * CH
    o["WO"] = c; c += 4 * COUT         # [128, 4, COUT]
    o["COS"] = c; c += T               # [128, T]
    o["SIN"] = c; c += T
    o["MASK"] = c; c += 128            # [128, 128]
    o["SELA"] = c; c += 8              # [128, 8]
    o["SELB"] = c; c += 8
    o["SELTA"] = c; c += 128           # rows 0:8 used, [8, 128]
    o["SELTB"] = c; c += 128
    o["NCOL"] = c
    return o


def qk_col_perm():
    """perm[j] = plain column (64*h + d) stored at device column j."""
    perm = np.zeros(CH, dtype=np.int64)
    for h in range(NH):
        for half in range(2):
            for r in range(32):
                j = 128 * (2 * (h // 4) + half) + 32 * (h % 4) + r
                perm[j] = 64 * h + 32 * half + r
    return perm


def make_consts(T):
    """Host-side constant tables (f32; rounded to bf16 at blob pack time)."""
    cs_d = D // 2
    inv_freq = 1.0 / (10000.0 ** (np.arange(cs_d, dtype=np.float64) / cs_d))
    freqs = np.outer(np.arange(T, dtype=np.float64), inv_freq)  # [T, 32]
    cosT = np.cos(freqs).astype(np.float32).T  # [32, T]
    sinT = np.sin(freqs).astype(np.float32).T
    COS = np.tile(cosT, (4, 1))  # [128, T]
    SIN = np.tile(sinT, (4, 1))
    p = np.arange(128)[:, None]
    j = np.arange(128)[None, :]
    MASK = (p <= j).astype(np.float32)
    SELA = np.zeros((128, 8), dtype=np.float32)
    SELB = np.zeros((128, 8), dtype=np.float32)
    for pp in range(128):
        SELA[pp, pp // 32] = 1.0
        SELB[pp, 4 + pp // 32] = 1.0
    return dict(COS=COS, SIN=SIN, MASK=MASK, SELA=SELA, SELB=SELB,
                SELTA=SELA.T.copy(), SELTB=SELB.T.copy())


def _chunked_rows(a):
    """[CIN, M] -> [128, CIN//128 * M]: row c = 128*ko + ki lands at
    partition ki, col chunk ko."""
    cin, m = a.shape
    kc = cin // 128
    return np.ascontiguousarray(
        a.reshape(kc, 128, m).transpose(1, 0, 2).reshape(128, kc * m))


def make_core_inputs(x_b, Wq_s, Wk_s, Wv_s, Wo_s, consts):
    """x_b [T, CIN]; W*_s this core's shards: Wq/Wk/Wv [CIN, 512] (plain
    column order 64h+d), Wo_s [512, COUT]. Returns {"BLOB": packed bf16}."""
    T = x_b.shape[0]
    cin = x_b.shape[1]
    cout = Wo_s.shape[1]
    o = blob_layout(T, cin, cout)
    perm = qk_col_perm()
    blob = np.zeros((128, o["NCOL"]), dtype=np.float32)
    blob[:, o["X"]:o["WQ"]] = _chunked_rows(np.ascontiguousarray(x_b.T))
    blob[:, o["WQ"]:o["WK"]] = _chunked_rows(Wq_s[:, perm])
    blob[:, o["WK"]:o["WV"]] = _chunked_rows(Wk_s[:, perm])
    blob[:, o["WV"]:o["WO"]] = _chunked_rows(Wv_s)
    blob[:, o["WO"]:o["COS"]] = _chunked_rows(Wo_s)
    blob[:, o["COS"]:o["SIN"]] = consts["COS"]
    blob[:, o["SIN"]:o["MASK"]] = consts["SIN"]
    blob[:, o["MASK"]:o["SELA"]] = consts["MASK"]
    blob[:, o["SELA"]:o["SELB"]] = consts["SELA"]
    blob[:, o["SELB"]:o["SELTA"]] = consts["SELB"]
    blob[0:8, o["SELTA"]:o["SELTB"]] = consts["SELTA"]
    blob[0:8, o["SELTB"]:o["NCOL"]] = consts["SELTB"]
    return {"BLOB": blob.astype(BF_NP)}


def build_nc(T, CIN, COUT):
    """Build the Bass program. T seq len, CIN input channels, COUT out channels."""
    assert T % 512 == 0 and CIN % 128 == 0 and COUT % 512 == 0
    KC = CIN // 128        # c_in chunks
    NTB = T // 512         # projection t-blocks == tq blocks
    NQ = T // 512
    NKC = T // 128         # tk chunks
    NCO = COUT // 512      # out-proj column halves
    o = blob_layout(T, CIN, COUT)

    nc = bacc.Bacc()
    BLOB = nc.dram_tensor("BLOB", [128, o["NCOL"]], BF16, kind="ExternalInput")
    OUT = nc.dram_tensor("OUT", [T, COUT], BF16, kind="ExternalOutput")

    with tile.TileContext(nc) as tc:
        with (
            tc.tile_pool(name="consts", bufs=1) as cpool,
            tc.tile_pool(name="big", bufs=1) as big,
            tc.tile_pool(name="work", bufs=2) as work,
            tc.tile_pool(name="tmp", bufs=2) as tmp,
            tc.tile_pool(name="psa", bufs=1, space="PSUM") as psa,
            tc.tile_pool(name="psb", bufs=1, space="PSUM") as psb,
            tc.tile_pool(name="psy", bufs=4, space="PSUM") as psy,
        ):
            # ---- loads from the blob, ordered so the first projection
            # matmuls start as soon as WQ + the first x chunk land ----
            xfull = big.tile([128, KC * T], BF16, tag="xfull")
            wtile = big.tile([128, 3 * KC * CH + 4 * COUT], BF16, tag="wtile")
            cstile = big.tile([128, 2 * T], BF16, tag="cstile")
            ctile = cpool.tile([128, o["NCOL"] - o["MASK"]], BF16, tag="ctile")
            nc.sync.dma_start(out=ctile, in_=BLOB[:, o["MASK"]:o["NCOL"]])
            nc.sync.dma_start(out=wtile[:, 0:KC * CH],
                              in_=BLOB[:, o["WQ"]:o["WK"]])
            for k in range(KC):
                nc.sync.dma_start(
                    out=xfull[:, k * T:(k + 1) * T],
                    in_=BLOB[:, o["X"] + k * T:o["X"] + (k + 1) * T])
            nc.sync.dma_start(out=cstile, in_=BLOB[:, o["COS"]:o["MASK"]])
            nc.sync.dma_start(out=wtile[:, KC * CH:2 * KC * CH],
                              in_=BLOB[:, o["WK"]:o["WV"]])
            nc.sync.dma_start(out=wtile[:, 2 * KC * CH:3 * KC * CH],
                              in_=BLOB[:, o["WV"]:o["WO"]])
            nc.sync.dma_start(out=wtile[:, 3 * KC * CH:],
                              in_=BLOB[:, o["WO"]:o["COS"]])

            def xs(k, lo, n):      # x chunk k, t-cols [lo, lo+n)
                return xfull[:, k * T + lo: k * T + lo + n]

            def ws(which, k, lo, n):   # weight section, chunk k, cols [lo,lo+n)
                base = {"WQ": 0, "WK": KC * CH, "WV": 2 * KC * CH,
                        "WO": 3 * KC * CH}[which]
                return wtile[:, base + k * CH + lo: base + k * CH + lo + n]

            def wo_s(m, lo, n):
                return wtile[:, 3 * KC * CH + m * COUT + lo:
                             3 * KC * CH + m * COUT + lo + n]

            mb = o["MASK"]
            mask_sb = ctile[:, 0:128]
            sela_sb = ctile[:, o["SELA"] - mb:o["SELB"] - mb]
            selb_sb = ctile[:, o["SELB"] - mb:o["SELTA"] - mb]
            selta_sb = ctile[0:8, o["SELTA"] - mb:o["SELTB"] - mb]
            seltb_sb = ctile[0:8, o["SELTB"] - mb:o["NCOL"] - mb]

            ones_sb = cpool.tile([128, 64], BF16, tag="ones")
            nc.vector.memset(ones_sb, 1.0)
            biasq = cpool.tile([8, 1], F32, tag="biasq")
            nc.vector.memset(biasq, 64.0 * EPS)
            biask = cpool.tile([8, 1], F32, tag="biask")
            nc.vector.memset(biask, EPS)

            khat = big.tile([128, 4, T], BF16, tag="khat")
            qhat = big.tile([128, 4, T], BF16, tag="qhat")
            vsb = big.tile([128, NKC, 520], BF16, tag="v")
            vsb4 = vsb.rearrange("p n (h e) -> p n h e", e=65)
            nc.vector.memset(vsb4[:, :, :, 64], 1.0)

            def vproj(tb, j):
                """One V-projection quarter; PE-only until the staging copy.
                Issued between dependent stretches to keep the PE busy."""
                vp = psy.tile([128, 512], F32, tag="y", name=f"vp{tb}_{j}")
                for k in range(KC):
                    nc.tensor.matmul(
                        vp,
                        xs(k, 512 * tb + 128 * j, 128),
                        ws("WV", k, 0, CH),
                        start=(k == 0), stop=(k == KC - 1),
                    )
                nc.scalar.activation(
                    vsb4[:, 4 * tb + j, :, 0:64],
                    vp.rearrange("p (h d) -> p h d", d=64), ACTF.Copy)

            def project_qk(which, tb, dst, is_q, fill_jobs=()):
                """Project t-block tb into dst[:, :, ts] with RoPE+RMS.
                fill_jobs: thunks issuing independent PE work right after the
                projection matmuls, to cover the ACT/DVE rms latency."""
                ts = slice(512 * tb, 512 * (tb + 1))
                cos_sb = cstile[:, 512 * tb:512 * (tb + 1)]
                sin_sb = cstile[:, T + 512 * tb:T + 512 * (tb + 1)]
                qpa = psa.tile([128, 2, 512], F32, tag="pa", name="qpa")
                qpb = psb.tile([128, 2, 512], F32, tag="pb", name="qpb")
                # k-outer: each x chunk feeds all four accumulation groups
                # the moment its DMA lands
                for k in range(KC):
                    for cc in range(4):
                        qp_t = qpa if cc < 2 else qpb
                        nc.tensor.matmul(
                            qp_t[:, cc % 2, :],
                            ws(which, k, 128 * cc, 128),
                            xs(k, 512 * tb, 512),
                            start=(k == 0), stop=(k == KC - 1),
                        )
                qs = tmp.tile([128, 4, 512], BF16, tag="qs", bufs=1)
                nc.scalar.activation(qs[:, 0:2, :], qpa, ACTF.Copy)
                nc.scalar.activation(qs[:, 2:4, :], qpb, ACTF.Copy)
                for job in fill_jobs:
                    job()
                # unscaled rope into dst (scaled afterwards, once rms known)
                u1 = qs[:, 0::2, :]
                u2 = qs[:, 1::2, :]
                cosb = cos_sb[:, None, :].to_broadcast([128, 2, 512])
                sinb = sin_sb[:, None, :].to_broadcast([128, 2, 512])
                e1 = tmp.tile([128, 2, 512], BF16, tag="r512", bufs=2)
                e2 = tmp.tile([128, 2, 512], BF16, tag="r512", bufs=2)
                nc.vector.tensor_mul(e1, u1, cosb)
                nc.vector.tensor_mul(e2, u2, sinb)
                nc.vector.tensor_add(dst[:, 0::2, ts], e1, e2)
                e3 = tmp.tile([128, 2, 512], BF16, tag="r512", bufs=2)
                e4 = tmp.tile([128, 2, 512], BF16, tag="r512", bufs=2)
                nc.vector.tensor_mul(e3, u2, cosb)
                nc.vector.tensor_mul(e4, u1, sinb)
                nc.vector.tensor_sub(dst[:, 1::2, ts], e3, e4)
                # per-head sum of squares (pre-rope == post-rope norms)
                qsq = tmp.tile([128, 4, 512], BF16, tag="qsq", bufs=2)
                nc.vector.tensor_mul(qsq, qs, qs)

                def finish():
                    # rms-scale matmuls, deferred until the next projection's
                    # main matmuls are queued so the PE never stalls on the
                    # ACT->DVE chain that produces qsq/rr
                    ssq = psy.tile([8, 512], F32, tag="y", name="ssq")
                    for cc in range(4):
                        nc.tensor.matmul(
                            ssq,
                            sela_sb if cc < 2 else selb_sb,
                            qsq[:, cc, :],
                            start=(cc == 0), stop=(cc == 3),
                        )
                    sq = tmp.tile([8, 512], F32, tag="sq")
                    if is_q:  # 1/sqrt(ssq + 64 eps): folds the 1/sqrt(D) scale
                        nc.scalar.activation(sq, ssq, ACTF.Sqrt, bias=biasq,
                                             scale=1.0)
                    else:     # 1/sqrt(ssq/64 + eps)
                        nc.scalar.activation(sq, ssq, ACTF.Sqrt, bias=biask,
                                             scale=1.0 / 64.0)
                    rr = tmp.tile([8, 512], BF16, tag="rr")
                    with nc.allow_low_precision(
                            reason="bf16 feed to PE broadcast"):
                        nc.vector.reciprocal(rr, sq)
                    for pr in range(2):
                        bq = psy.tile([128, 512], F32, tag="y", name=f"bq{pr}")
                        nc.tensor.matmul(
                            bq, selta_sb if pr == 0 else seltb_sb, rr,
                            start=True, stop=True,
                        )
                        bqs = tmp.tile([128, 512], BF16, tag="bqs", bufs=2)
                        nc.scalar.activation(bqs, bq, ACTF.Copy)
                        nc.vector.tensor_mul(
                            dst[:, 2 * pr:2 * pr + 2, ts],
                            dst[:, 2 * pr:2 * pr + 2, ts],
                            bqs[:, None, :].to_broadcast([128, 2, 512]),
                        )
                return finish

            # ============ Phase A: Q-hat, K-hat, V (per t-block) =========
            fink = None
            for tb in range(NTB):
                finq = project_qk("WQ", tb, qhat, is_q=True,
                                  fill_jobs=[lambda t=tb: vproj(t, 0),
                                             lambda t=tb: vproj(t, 1)])
                if fink is not None:
                    fink()  # k(tb-1) rms scale, inputs long ready
                fink = project_qk("WK", tb, khat, is_q=False,
                                  fill_jobs=[lambda t=tb: vproj(t, 2),
                                             lambda t=tb: vproj(t, 3)])
                finq()      # q(tb) rms scale, covered by the k matmuls
            fink()          # k(NTB-1); phase B's first chunks read khat
                            # t-block 0, so this tail overlaps phase B

            # ============ Phase B: per tq block: attention + out-proj ====
            for qb in range(NQ):
                yhat = work.tile([128, 4, 512], BF16, tag="yhat", bufs=1)
                for g in range(2):
                    ybank = [psy.tile([65, 512], F32, tag="y",
                                      name=f"y{qb}_{g}_{j_}") for j_ in range(4)]
                    nkc = 4 * (qb + 1)
                    def issue_scores(c, pj):
                        # for diagonal chunks only columns [128*kd, 512)
                        # are live: scores, exp, mask, and PV all restrict
                        # to the suffix. (chunk 0 is always full-width.)
                        kd = c - 4 * qb
                        lo = 128 * kd if kd > 0 else 0
                        pool = psa if pj == 0 else psb
                        sc = pool.tile([128, 2, 512], F32,
                                       tag=("pa" if pj == 0 else "pb"),
                                       name=f"sc{pj}")
                        for e in range(2):
                            j = 2 * pj + e
                            for half in range(2):
                                cc = 2 * g + half
                                nc.tensor.matmul(
                                    sc[:, e, lo:],
                                    khat[32 * j:32 * (j + 1), cc,
                                         128 * c:128 * (c + 1)],
                                    qhat[32 * j:32 * (j + 1), cc,
                                         512 * qb + lo:512 * (qb + 1)],
                                    start=(half == 0), stop=(half == 1),
                                    tile_position=(32 * j, 0),
                                )
                        return sc

                    def issue_pv(c, pj, sc):
                        kd = c - 4 * qb
                        lo = 128 * kd if kd > 0 else 0
                        first, last = (c == 0), (c == nkc - 1)
                        ph = tmp.tile([128, 2, 512], BF16, tag="r512",
                                      bufs=2, name=f"ph{pj}")
                        nc.scalar.activation(
                            ph[:, :, lo:], sc[:, :, lo:], ACTF.Exp)
                        if kd >= 0:  # diagonal boundary strip
                            nc.vector.tensor_mul(
                                ph[:, :, 128 * kd:128 * (kd + 1)],
                                ph[:, :, 128 * kd:128 * (kd + 1)],
                                mask_sb[:, None, :].to_broadcast(
                                    [128, 2, 128]),
                            )
                        for e in range(2):
                            j = 2 * pj + e
                            hloc = 4 * g + j
                            nc.tensor.matmul(
                                ybank[j][:, lo:],
                                vsb[:, c, 65 * hloc:65 * hloc + 65],
                                ph[:, e, lo:],
                                start=first, stop=last,
                                skip_group_check=True,
                            )

                    def issue_rcp(j):
                        # reciprocal of ybank[j]'s denominator row, issued
                        # the moment its accumulation group stops so the rb
                        # broadcast matmuls never wait on the DVE queue
                        rcp = tmp.tile([128, 512], BF16, tag="s512", bufs=4)
                        with nc.allow_low_precision(reason="bf16 for PE bcast"):
                            nc.vector.reciprocal(rcp[64:65, :],
                                                 ybank[j][64:65, :])
                        return rcp

                    # Software pipeline by one (chunk, pair) stage: the PE
                    # streams unit i+1's score matmuls while ACT/DVE turn
                    # unit i's scores into probabilities.
                    units = [(c, pj) for c in range(nkc) for pj in range(2)]
                    rcps = {}
                    pending = [issue_scores(*units[0])]
                    for i, (c, pj) in enumerate(units):
                        if i + 1 < len(units):
                            pending.append(issue_scores(*units[i + 1]))
                        issue_pv(c, pj, pending.pop(0))
                        if c == nkc - 1:  # this pair's ybanks just stopped
                            rcps[2 * pj] = issue_rcp(2 * pj)
                            rcps[2 * pj + 1] = issue_rcp(2 * pj + 1)
                    # normalize: yhat rows = y / denom. Banks alternate and
                    # each copy issues right after its matmul, so rb(j+2)
                    # only waits on the (long done) rbs(j) copy.
                    for j in range(4):
                        hloc = 4 * g + j
                        pool = psa if j % 2 == 0 else psb
                        rb = pool.tile([128, 512], F32,
                                       tag=("pa" if j % 2 == 0 else "pb"),
                                       name=f"rb{qb}_{g}_{j}")
                        nc.tensor.matmul(
                            rb[0:64, :],
                            ones_sb[64:65, :],
                            rcps[j][64:65, :],
                            start=True, stop=True,
                            tile_position=(64, 0),
                            skip_group_check=True,
                        )
                        rbs = tmp.tile([128, 512], F32, tag="s512f", bufs=2)
                        # Copy staged on ACT (idle after the last exp) so the
                        # DVE only runs the rcps and yhat muls here.
                        nc.scalar.activation(rbs[0:64, :], rb[0:64, :],
                                             ACTF.Copy)
                        nc.vector.tensor_mul(
                            yhat[64 * (hloc % 2):64 * (hloc % 2 + 1),
                                 hloc // 2, :],
                            ybank[j][0:64, :],
                            rbs[0:64, :],
                        )
                # out projection for this tq block
                for jt in range(4):
                    osb = tmp.tile([128, COUT], BF16, tag="osb", bufs=2)
                    for n in range(NCO):
                        op = psy.tile([128, 512], F32, tag="y",
                                      name=f"op{qb}_{n}_{jt}")
                        for m in range(4):
                            nc.tensor.matmul(
                                op,
                                yhat[:, m, 128 * jt:128 * (jt + 1)],
                                wo_s(m, 512 * n, 512),
                                start=(m == 0), stop=(m == 3),
                            )
                        nc.vector.tensor_copy(out=osb[:, 512 * n:512 * (n + 1)],
                                              in_=op)
                    nc.sync.dma_start(
                        out=OUT[512 * qb + 128 * jt:512 * qb + 128 * (jt + 1), :],
                        in_=osb)

    nc.finalize()
    return nc


# ======================================================================
# Full-problem harness: 8 cores = 4 batch x 2 head-groups
# ======================================================================
B_FULL, T_FULL, C_FULL, H_FULL = 4, 2048, 1024, 16

_NC_CACHE = {}


def _get_nc():
    if "nc" not in _NC_CACHE:
        _NC_CACHE["nc"] = build_nc(T_FULL, C_FULL, C_FULL)
    return _NC_CACHE["nc"]


def _consts_from_tables(cos, sin):
    """Like make_consts but using the provided RoPE tables.
    cos/sin: [1, 1, T, 32] float32."""
    c = make_consts(T_FULL)
    c["COS"] = np.ascontiguousarray(np.tile(np.asarray(cos)[0, 0].T, (4, 1)))
    c["SIN"] = np.ascontiguousarray(np.tile(np.asarray(sin)[0, 0].T, (4, 1)))
    return c


def make_in_maps(x, cos, sin, Wq, Wk, Wv, Wo):
    x, Wq, Wk, Wv, Wo = (np.asarray(a, dtype=np.float32)
                         for a in (x, Wq, Wk, Wv, Wo))
    consts = _consts_from_tables(cos, sin)
    in_maps = []
    for core in range(8):
        b, hg = core // 2, core % 2
        cols = slice(512 * hg, 512 * (hg + 1))
        in_maps.append(make_core_inputs(
            x[b], Wq[:, cols], Wk[:, cols], Wv[:, cols], Wo[cols, :], consts))
    return in_maps


def gather_out(results):
    out = np.empty((B_FULL, T_FULL, C_FULL), dtype=np.float32)
    for b in range(B_FULL):
        out[b] = (results[2 * b]["OUT"].astype(np.float32)
                  + results[2 * b + 1]["OUT"].astype(np.float32))
    return out


def kernel(x, cos, sin, Wq, Wk, Wv, Wo):
    from concourse.bass_utils import run_bass_kernel_spmd
    nc = _get_nc()
    in_maps = make_in_maps(x, cos, sin, Wq, Wk, Wv, Wo)
    res = run_bass_kernel_spmd(nc, in_maps, core_ids=list(range(8)))
    return gather_out(res.results)


# revision 19
# speedup vs baseline: 11.4000x; 1.2799x over previous
"""Causal self-attention kernel (RoPE + parameter-free RMSNorm on Q/K).

Sharding: 8 cores = 4 batch x 2 head-groups (8 heads each). Each core computes
its batch element's attention for its 8 heads plus the partial output
projection; host sums the two head-group partials per batch element.

All per-core inputs ship as ONE packed bf16 blob [128, NCOL] (x, weights, RoPE
tables, mask/select constants) to minimize per-call host->device traffic and
buffer-binding overhead on the axon exec path. All matmuls run bf16 with f32
PSUM accumulation; the output ships back as bf16 partials summed on host in
f32. Q-hat stays resident in SBUF (no DRAM round trip).

Per-core device layout (D=64, 8 heads):
  Q^T / K^T stored as [128, 4, T]: col j = 128*cc + p,
     cc = 2*(h//4) + half, p = 32*(h%4) + r,  (d = 32*half + r)
  V stored with a ones column per head: [128, T//128, 8*65]; the ones column
  makes the PV matmul also accumulate the softmax denominator (row 64).
  Scores computed transposed: S^T[tk, tq] per head via K=32 row-tiled matmuls;
  softmax runs without max-subtraction (RMS-normed q,k bound |s| <= 8);
  the denominator division folds in before the output projection.
"""

import sys

import numpy as np
import ml_dtypes

for _p in ("/opt/trn_rl_repo",):
    if _p not in sys.path:
        sys.path.insert(0, _p)

import concourse.bass as bass
import concourse.mybir as mybir
import concourse.tile as tile
from concourse import bacc

F32 = mybir.dt.float32
BF16 = mybir.dt.bfloat16
AX = mybir.AluOpType
ACTF = mybir.ActivationFunctionType
BF_NP = ml_dtypes.bfloat16

D = 64
NH = 8          # heads per core
CH = NH * D     # 512 head channels per core
EPS = float(np.finfo(np.float32).eps)


def blob_layout(T, CIN=1024, COUT=1024):
    """Column offsets of each section in the packed [128, NCOL] bf16 blob."""
    KC = CIN // 128
    o = {}
    c = 0
    o["X"] = c; c += KC * T            # [128, KC, T]
    o["WQ"] = c; c += KC * CH          # [128, KC, 512]
    o["WK"] = c; c += KC * CH
    o["WV"] = c; c += KC * CH
    o["WO"] = c; c += 4 * COUT         # [128, 4, COUT]
    # CST packs the 32-row RoPE tables and the 8-row SELT matrices
    # vertically: rows 0:32 cos, 32:64 sin, 64:72 SELTA, 72:80 SELTB.
    # The device replicates cos/sin x4 across partition groups on load.
    o["CST"] = c; c += T
    o["MASK"] = c; c += 128            # [128, 128]
    o["SELA"] = c; c += 8              # [128, 8]
    o["SELB"] = c; c += 8
    o["NCOL"] = c
    return o


def qk_col_perm():
    """perm[j] = plain column (64*h + d) stored at device column j."""
    perm = np.zeros(CH, dtype=np.int64)
    for h in range(NH):
        for half in range(2):
            for r in range(32):
                j = 128 * (2 * (h // 4) + half) + 32 * (h % 4) + r
                perm[j] = 64 * h + 32 * half + r
    return perm


def make_consts(T):
    """Host-side constant tables (f32; rounded to bf16 at blob pack time)."""
    cs_d = D // 2
    inv_freq = 1.0 / (10000.0 ** (np.arange(cs_d, dtype=np.float64) / cs_d))
    freqs = np.outer(np.arange(T, dtype=np.float64), inv_freq)  # [T, 32]
    cosT = np.cos(freqs).astype(np.float32).T  # [32, T]
    sinT = np.sin(freqs).astype(np.float32).T
    COS = np.tile(cosT, (4, 1))  # [128, T]
    SIN = np.tile(sinT, (4, 1))
    p = np.arange(128)[:, None]
    j = np.arange(128)[None, :]
    MASK = (p <= j).astype(np.float32)
    SELA = np.zeros((128, 8), dtype=np.float32)
    SELB = np.zeros((128, 8), dtype=np.float32)
    for pp in range(128):
        SELA[pp, pp // 32] = 1.0
        SELB[pp, 4 + pp // 32] = 1.0
    return dict(COS=COS, SIN=SIN, MASK=MASK, SELA=SELA, SELB=SELB,
                SELTA=SELA.T.copy(), SELTB=SELB.T.copy())


def _chunked_rows(a):
    """[CIN, M] -> [128, CIN//128 * M]: row c = 128*ko + ki lands at
    partition ki, col chunk ko."""
    cin, m = a.shape
    kc = cin // 128
    return np.ascontiguousarray(
        a.reshape(kc, 128, m).transpose(1, 0, 2).reshape(128, kc * m))


def make_core_inputs(x_b, Wq_s, Wk_s, Wv_s, Wo_s, consts):
    """x_b [T, CIN]; W*_s this core's shards: Wq/Wk/Wv [CIN, 512] (plain
    column order 64h+d), Wo_s [512, COUT]. Returns {"BLOB": packed bf16}."""
    T = x_b.shape[0]
    cin = x_b.shape[1]
    cout = Wo_s.shape[1]
    o = blob_layout(T, cin, cout)
    perm = qk_col_perm()
    blob = np.zeros((128, o["NCOL"]), dtype=np.float32)
    blob[:, o["X"]:o["WQ"]] = _chunked_rows(np.ascontiguousarray(x_b.T))
    blob[:, o["WQ"]:o["WK"]] = _chunked_rows(Wq_s[:, perm])
    blob[:, o["WK"]:o["WV"]] = _chunked_rows(Wk_s[:, perm])
    blob[:, o["WV"]:o["WO"]] = _chunked_rows(Wv_s)
    blob[:, o["WO"]:o["CST"]] = _chunked_rows(Wo_s)
    blob[0:32, o["CST"]:o["MASK"]] = consts["COS"][0:32]
    blob[32:64, o["CST"]:o["MASK"]] = consts["SIN"][0:32]
    blob[64:72, o["CST"]:o["CST"] + 128] = consts["SELTA"]
    blob[72:80, o["CST"]:o["CST"] + 128] = consts["SELTB"]
    blob[:, o["MASK"]:o["SELA"]] = consts["MASK"]
    blob[:, o["SELA"]:o["SELB"]] = consts["SELA"]
    blob[:, o["SELB"]:o["NCOL"]] = consts["SELB"]
    return {"BLOB": blob.astype(BF_NP)}


def build_nc(T, CIN, COUT):
    """Build the Bass program. T seq len, CIN input channels, COUT out channels."""
    assert T % 512 == 0 and CIN % 128 == 0 and COUT % 512 == 0
    KC = CIN // 128        # c_in chunks
    NTB = T // 512         # projection t-blocks == tq blocks
    NQ = T // 512
    NKC = T // 128         # tk chunks
    NCO = COUT // 512      # out-proj column halves
    o = blob_layout(T, CIN, COUT)

    nc = bacc.Bacc()
    BLOB = nc.dram_tensor("BLOB", [128, o["NCOL"]], BF16, kind="ExternalInput")
    OUT = nc.dram_tensor("OUT", [T, COUT], BF16, kind="ExternalOutput")

    with tile.TileContext(nc) as tc:
        with (
            tc.tile_pool(name="consts", bufs=1) as cpool,
            tc.tile_pool(name="big", bufs=1) as big,
            tc.tile_pool(name="work", bufs=2) as work,
            tc.tile_pool(name="tmp", bufs=2) as tmp,
            tc.tile_pool(name="psa", bufs=1, space="PSUM") as psa,
            tc.tile_pool(name="psb", bufs=1, space="PSUM") as psb,
            tc.tile_pool(name="psy", bufs=4, space="PSUM") as psy,
        ):
            # ---- loads from the blob, ordered so the first projection
            # matmuls start as soon as WQ + the first x chunk land ----
            xfull = big.tile([128, KC * T], BF16, tag="xfull")
            wtile = big.tile([128, 3 * KC * CH + 4 * COUT], BF16, tag="wtile")
            cstile = big.tile([128, 2 * T], BF16, tag="cstile")
            ctile = cpool.tile([128, o["NCOL"] - o["MASK"]], BF16, tag="ctile")
            nc.sync.dma_start(out=ctile, in_=BLOB[:, o["MASK"]:o["NCOL"]])
            selt = cpool.tile([8, 256], BF16, tag="selt")
            nc.sync.dma_start(out=selt[0:8, 0:128],
                              in_=BLOB[64:72, o["CST"]:o["CST"] + 128])
            nc.sync.dma_start(out=selt[0:8, 128:256],
                              in_=BLOB[72:80, o["CST"]:o["CST"] + 128])
            nc.sync.dma_start(out=wtile[:, 0:KC * CH],
                              in_=BLOB[:, o["WQ"]:o["WK"]])
            for k in range(KC):
                nc.sync.dma_start(
                    out=xfull[:, k * T:(k + 1) * T],
                    in_=BLOB[:, o["X"] + k * T:o["X"] + (k + 1) * T])
            # replicate the 32-row cos/sin tables across the 4 partition
            # groups (the blob ships them once)
            for g4 in range(4):
                nc.sync.dma_start(
                    out=cstile[32 * g4:32 * (g4 + 1), 0:T],
                    in_=BLOB[0:32, o["CST"]:o["CST"] + T])
                nc.sync.dma_start(
                    out=cstile[32 * g4:32 * (g4 + 1), T:2 * T],
                    in_=BLOB[32:64, o["CST"]:o["CST"] + T])
            nc.sync.dma_start(out=wtile[:, KC * CH:2 * KC * CH],
                              in_=BLOB[:, o["WK"]:o["WV"]])
            nc.sync.dma_start(out=wtile[:, 2 * KC * CH:3 * KC * CH],
                              in_=BLOB[:, o["WV"]:o["WO"]])
            nc.sync.dma_start(out=wtile[:, 3 * KC * CH:],
                              in_=BLOB[:, o["WO"]:o["CST"]])

            def xs(k, lo, n):      # x chunk k, t-cols [lo, lo+n)
                return xfull[:, k * T + lo: k * T + lo + n]

            def ws(which, k, lo, n):   # weight section, chunk k, cols [lo,lo+n)
                base = {"WQ": 0, "WK": KC * CH, "WV": 2 * KC * CH,
                        "WO": 3 * KC * CH}[which]
                return wtile[:, base + k * CH + lo: base + k * CH + lo + n]

            def wo_s(m, lo, n):
                return wtile[:, 3 * KC * CH + m * COUT + lo:
                             3 * KC * CH + m * COUT + lo + n]

            mb = o["MASK"]
            mask_sb = ctile[:, 0:128]
            sela_sb = ctile[:, o["SELA"] - mb:o["SELB"] - mb]
            selb_sb = ctile[:, o["SELB"] - mb:o["NCOL"] - mb]
            selta_sb = selt[0:8, 0:128]
            seltb_sb = selt[0:8, 128:256]

            ones_sb = cpool.tile([128, 64], BF16, tag="ones")
            nc.vector.memset(ones_sb, 1.0)
            biasq = cpool.tile([8, 1], F32, tag="biasq")
            nc.vector.memset(biasq, 64.0 * EPS)
            biask = cpool.tile([8, 1], F32, tag="biask")
            nc.vector.memset(biask, EPS)

            khat = big.tile([128, 4, T], BF16, tag="khat")
            qhat = big.tile([128, 4, T], BF16, tag="qhat")
            vsb = big.tile([128, NKC, 520], BF16, tag="v")
            vsb4 = vsb.rearrange("p n (h e) -> p n h e", e=65)
            nc.vector.memset(vsb4[:, :, :, 64], 1.0)

            def vproj(tb, j):
                """One V-projection quarter; PE-only until the staging copy.
                Issued between dependent stretches to keep the PE busy."""
                vp = psy.tile([128, 512], F32, tag="y", name=f"vp{tb}_{j}")
                for k in range(KC):
                    nc.tensor.matmul(
                        vp,
                        xs(k, 512 * tb + 128 * j, 128),
                        ws("WV", k, 0, CH),
                        start=(k == 0), stop=(k == KC - 1),
                    )
                nc.scalar.activation(
                    vsb4[:, 4 * tb + j, :, 0:64],
                    vp.rearrange("p (h d) -> p h d", d=64), ACTF.Copy)

            def project_qk(which, tb, dst, is_q, fill_jobs=()):
                """Project t-block tb into dst[:, :, ts] with RoPE+RMS.
                fill_jobs: thunks issuing independent PE work right after the
                projection matmuls, to cover the ACT/DVE rms latency."""
                ts = slice(512 * tb, 512 * (tb + 1))
                cos_sb = cstile[:, 512 * tb:512 * (tb + 1)]
                sin_sb = cstile[:, T + 512 * tb:T + 512 * (tb + 1)]
                qpa = psa.tile([128, 2, 512], F32, tag="pa", name="qpa")
                qpb = psb.tile([128, 2, 512], F32, tag="pb", name="qpb")
                # k-outer: each x chunk feeds all four accumulation groups
                # the moment its DMA lands
                for k in range(KC):
                    for cc in range(4):
                        qp_t = qpa if cc < 2 else qpb
                        nc.tensor.matmul(
                            qp_t[:, cc % 2, :],
                            ws(which, k, 128 * cc, 128),
                            xs(k, 512 * tb, 512),
                            start=(k == 0), stop=(k == KC - 1),
                        )
                qs = tmp.tile([128, 4, 512], BF16, tag="qs", bufs=1)
                nc.scalar.activation(qs[:, 0:2, :], qpa, ACTF.Copy)
                nc.scalar.activation(qs[:, 2:4, :], qpb, ACTF.Copy)
                for job in fill_jobs:
                    job()
                # unscaled rope into dst (scaled afterwards, once rms known)
                u1 = qs[:, 0::2, :]
                u2 = qs[:, 1::2, :]
                cosb = cos_sb[:, None, :].to_broadcast([128, 2, 512])
                sinb = sin_sb[:, None, :].to_broadcast([128, 2, 512])
                e1 = tmp.tile([128, 2, 512], BF16, tag="r512", bufs=2)
                e2 = tmp.tile([128, 2, 512], BF16, tag="r512", bufs=2)
                nc.vector.tensor_mul(e1, u1, cosb)
                nc.vector.tensor_mul(e2, u2, sinb)
                nc.vector.tensor_add(dst[:, 0::2, ts], e1, e2)
                e3 = tmp.tile([128, 2, 512], BF16, tag="r512", bufs=2)
                e4 = tmp.tile([128, 2, 512], BF16, tag="r512", bufs=2)
                nc.vector.tensor_mul(e3, u2, cosb)
                nc.vector.tensor_mul(e4, u1, sinb)
                nc.vector.tensor_sub(dst[:, 1::2, ts], e3, e4)
                # per-head sum of squares (pre-rope == post-rope norms)
                qsq = tmp.tile([128, 4, 512], BF16, tag="qsq", bufs=2)
                nc.vector.tensor_mul(qsq, qs, qs)

                def finish():
                    # rms-scale matmuls, deferred until the next projection's
                    # main matmuls are queued so the PE never stalls on the
                    # ACT->DVE chain that produces qsq/rr
                    ssq = psy.tile([8, 512], F32, tag="y", name="ssq")
                    for cc in range(4):
                        nc.tensor.matmul(
                            ssq,
                            sela_sb if cc < 2 else selb_sb,
                            qsq[:, cc, :],
                            start=(cc == 0), stop=(cc == 3),
                        )
                    sq = tmp.tile([8, 512], F32, tag="sq")
                    if is_q:  # 1/sqrt(ssq + 64 eps): folds the 1/sqrt(D) scale
                        nc.scalar.activation(sq, ssq, ACTF.Sqrt, bias=biasq,
                                             scale=1.0)
                    else:     # 1/sqrt(ssq/64 + eps)
                        nc.scalar.activation(sq, ssq, ACTF.Sqrt, bias=biask,
                                             scale=1.0 / 64.0)
                    rr = tmp.tile([8, 512], BF16, tag="rr")
                    with nc.allow_low_precision(
                            reason="bf16 feed to PE broadcast"):
                        nc.vector.reciprocal(rr, sq)
                    for pr in range(2):
                        bq = psy.tile([128, 512], F32, tag="y", name=f"bq{pr}")
                        nc.tensor.matmul(
                            bq, selta_sb if pr == 0 else seltb_sb, rr,
                            start=True, stop=True,
                        )
                        bqs = tmp.tile([128, 512], BF16, tag="bqs", bufs=2)
                        nc.scalar.activation(bqs, bq, ACTF.Copy)
                        nc.vector.tensor_mul(
                            dst[:, 2 * pr:2 * pr + 2, ts],
                            dst[:, 2 * pr:2 * pr + 2, ts],
                            bqs[:, None, :].to_broadcast([128, 2, 512]),
                        )
                return finish

            # ============ Phase A: Q-hat, K-hat, V (per t-block) =========
            fink = None
            for tb in range(NTB):
                finq = project_qk("WQ", tb, qhat, is_q=True,
                                  fill_jobs=[lambda t=tb: vproj(t, 0),
                                             lambda t=tb: vproj(t, 1)])
                if fink is not None:
                    fink()  # k(tb-1) rms scale, inputs long ready
                fink = project_qk("WK", tb, khat, is_q=False,
                                  fill_jobs=[lambda t=tb: vproj(t, 2),
                                             lambda t=tb: vproj(t, 3)])
                finq()      # q(tb) rms scale, covered by the k matmuls
            fink()          # k(NTB-1); phase B's first chunks read khat
                            # t-block 0, so this tail overlaps phase B

            # ============ Phase B: per tq block: attention + out-proj ====
            for qb in range(NQ):
                yhat = work.tile([128, 4, 512], BF16, tag="yhat", bufs=1)
                for g in range(2):
                    ybank = [psy.tile([65, 512], F32, tag="y",
                                      name=f"y{qb}_{g}_{j_}") for j_ in range(4)]
                    nkc = 4 * (qb + 1)
                    def issue_scores(c, pj):
                        # for diagonal chunks only columns [128*kd, 512)
                        # are live: scores, exp, mask, and PV all restrict
                        # to the suffix. (chunk 0 is always full-width.)
                        kd = c - 4 * qb
                        lo = 128 * kd if kd > 0 else 0
                        pool = psa if pj == 0 else psb
                        sc = pool.tile([128, 2, 512], F32,
                                       tag=("pa" if pj == 0 else "pb"),
                                       name=f"sc{pj}")
                        for e in range(2):
                            j = 2 * pj + e
                            for half in range(2):
                                cc = 2 * g + half
                                nc.tensor.matmul(
                                    sc[:, e, lo:],
                                    khat[32 * j:32 * (j + 1), cc,
                                         128 * c:128 * (c + 1)],
                                    qhat[32 * j:32 * (j + 1), cc,
                                         512 * qb + lo:512 * (qb + 1)],
                                    start=(half == 0), stop=(half == 1),
                                    tile_position=(32 * j, 0),
                                )
                        return sc

                    def issue_pv(c, pj, sc):
                        kd = c - 4 * qb
                        lo = 128 * kd if kd > 0 else 0
                        first, last = (c == 0), (c == nkc - 1)
                        ph = tmp.tile([128, 2, 512], BF16, tag="r512",
                                      bufs=2, name=f"ph{pj}")
                        nc.scalar.activation(
                            ph[:, :, lo:], sc[:, :, lo:], ACTF.Exp)
                        if kd >= 0:  # diagonal boundary strip
                            nc.vector.tensor_mul(
                                ph[:, :, 128 * kd:128 * (kd + 1)],
                                ph[:, :, 128 * kd:128 * (kd + 1)],
                                mask_sb[:, None, :].to_broadcast(
                                    [128, 2, 128]),
                            )
                        for e in range(2):
                            j = 2 * pj + e
                            hloc = 4 * g + j
                            nc.tensor.matmul(
                                ybank[j][:, lo:],
                                vsb[:, c, 65 * hloc:65 * hloc + 65],
                                ph[:, e, lo:],
                                start=first, stop=last,
                                skip_group_check=True,
                            )

                    def issue_rcp(j):
                        # reciprocal of ybank[j]'s denominator row, issued
                        # the moment its accumulation group stops so the rb
                        # broadcast matmuls never wait on the DVE queue
                        rcp = tmp.tile([128, 512], BF16, tag="s512", bufs=4)
                        with nc.allow_low_precision(reason="bf16 for PE bcast"):
                            nc.vector.reciprocal(rcp[64:65, :],
                                                 ybank[j][64:65, :])
                        return rcp

                    # Software pipeline by one (chunk, pair) stage: the PE
                    # streams unit i+1's score matmuls while ACT/DVE turn
                    # unit i's scores into probabilities.
                    units = [(c, pj) for c in range(nkc) for pj in range(2)]
                    rcps = {}
                    pending = [issue_scores(*units[0])]
                    for i, (c, pj) in enumerate(units):
                        if i + 1 < len(units):
                            pending.append(issue_scores(*units[i + 1]))
                        issue_pv(c, pj, pending.pop(0))
                        if c == nkc - 1:  # this pair's ybanks just stopped
                            rcps[2 * pj] = issue_rcp(2 * pj)
                            rcps[2 * pj + 1] = issue_rcp(2 * pj + 1)
                    # normalize: yhat rows = y / denom. Banks alternate and
                    # each copy issues right after its matmul, so rb(j+2)
                    # only waits on the (long done) rbs(j) copy.
                    for j in range(4):
                        hloc = 4 * g + j
                        pool = psa if j % 2 == 0 else psb
                        rb = pool.tile([128, 512], F32,
                                       tag=("pa" if j % 2 == 0 else "pb"),
                                       name=f"rb{qb}_{g}_{j}")
                        nc.tensor.matmul(
                            rb[0:64, :],
                            ones_sb[64:65, :],
                            rcps[j][64:65, :],
                            start=True, stop=True,
                            tile_position=(64, 0),
                            skip_group_check=True,
                        )
                        rbs = tmp.tile([128, 512], F32, tag="s512f", bufs=2)
                        # Copy staged on ACT (idle after the last exp) so the
                        # DVE only runs the rcps and yhat muls here.
                        nc.scalar.activation(rbs[0:64, :], rb[0:64, :],
                                             ACTF.Copy)
                        nc.vector.tensor_mul(
                            yhat[64 * (hloc % 2):64 * (hloc % 2 + 1),
                                 hloc // 2, :],
                            ybank[j][0:64, :],
                            rbs[0:64, :],
                        )
                # out projection for this tq block
                for jt in range(4):
                    osb = tmp.tile([128, COUT], BF16, tag="osb", bufs=2)
                    for n in range(NCO):
                        op = psy.tile([128, 512], F32, tag="y",
                                      name=f"op{qb}_{n}_{jt}")
                        for m in range(4):
                            nc.tensor.matmul(
                                op,
                                yhat[:, m, 128 * jt:128 * (jt + 1)],
                                wo_s(m, 512 * n, 512),
                                start=(m == 0), stop=(m == 3),
                            )
                        nc.vector.tensor_copy(out=osb[:, 512 * n:512 * (n + 1)],
                                              in_=op)
                    nc.sync.dma_start(
                        out=OUT[512 * qb + 128 * jt:512 * qb + 128 * (jt + 1), :],
                        in_=osb)

    nc.finalize()
    return nc


# ======================================================================
# Full-problem harness: 8 cores = 4 batch x 2 head-groups
# ======================================================================
B_FULL, T_FULL, C_FULL, H_FULL = 4, 2048, 1024, 16

_NC_CACHE = {}


def _get_nc():
    if "nc" not in _NC_CACHE:
        _NC_CACHE["nc"] = build_nc(T_FULL, C_FULL, C_FULL)
    return _NC_CACHE["nc"]


def _consts_from_tables(cos, sin):
    """Like make_consts but using the provided RoPE tables.
    cos/sin: [1, 1, T, 32] float32."""
    c = make_consts(T_FULL)
    c["COS"] = np.ascontiguousarray(np.tile(np.asarray(cos)[0, 0].T, (4, 1)))
    c["SIN"] = np.ascontiguousarray(np.tile(np.asarray(sin)[0, 0].T, (4, 1)))
    return c


def make_in_maps(x, cos, sin, Wq, Wk, Wv, Wo):
    x, Wq, Wk, Wv, Wo = (np.asarray(a, dtype=np.float32)
                         for a in (x, Wq, Wk, Wv, Wo))
    consts = _consts_from_tables(cos, sin)
    in_maps = []
    for core in range(8):
        b, hg = core // 2, core % 2
        cols = slice(512 * hg, 512 * (hg + 1))
        in_maps.append(make_core_inputs(
            x[b], Wq[:, cols], Wk[:, cols], Wv[:, cols], Wo[cols, :], consts))
    return in_maps


def gather_out(results):
    out = np.empty((B_FULL, T_FULL, C_FULL), dtype=np.float32)
    for b in range(B_FULL):
        out[b] = (results[2 * b]["OUT"].astype(np.float32)
                  + results[2 * b + 1]["OUT"].astype(np.float32))
    return out


def kernel(x, cos, sin, Wq, Wk, Wv, Wo):
    from concourse.bass_utils import run_bass_kernel_spmd
    nc = _get_nc()
    in_maps = make_in_maps(x, cos, sin, Wq, Wk, Wv, Wo)
    res = run_bass_kernel_spmd(nc, in_maps, core_ids=list(range(8)))
    return gather_out(res.results)
